# revision 1
# baseline (speedup 1.0000x reference)
"""Grouped-experts MoE FFN (SwiGLU) kernel for Trainium2, expert-parallel on 8 cores.

E=8 experts, D=2048, H=5632, T=32768 tokens pre-sorted by expert.
Each NeuronCore owns one expert and its token shard (padded to 4096 tokens).

Per-core dataflow (features on partitions, tokens on the free axis):
  h1T = w1T.T-accum over D:  psum[h,t] += w1T[d,h].T @ xT[d,t]
  h3T likewise; h = silu(h1)*h3 in SBUF (bf16)
  outT[dout,t] += w2T[h,dout].T @ h[h,t]  accumulated over all 44 h-tiles.
Token blocks of 1024 keep the h intermediate resident in SBUF (no DRAM spill);
w1/w3/w2 are re-streamed per block (~310 MB/core total, far under HBM roofline
for the ~3.6 ms of PE-bound compute).
"""

import sys

sys.path.insert(0, "/opt/trn_rl_repo")

import ml_dtypes
import numpy as np

import concourse.bass as bass  # noqa: F401
import concourse.mybir as mybir
import concourse.tile as tile
from concourse import bacc
from concourse.bass_utils import run_bass_kernel_spmd

BF16 = ml_dtypes.bfloat16

E, D, H, T = 8, 2048, 5632, 32768
N_CORES = 8
TPC = T // E  # tokens per core (4096), also the padded shard size


def _build(d=D, h=H, tpc=TPC, tb=1024, tc=512):
    """Build the Bass program (same program for all 8 cores; data differs)."""
    kd = d // 128
    kh = h // 128
    nc = bacc.Bacc("TRN2", target_bir_lowering=False, debug=False)

    xT = nc.dram_tensor("xT", [d, tpc], mybir.dt.bfloat16, kind="ExternalInput")
    w1t = nc.dram_tensor("w1t", [d, h], mybir.dt.bfloat16, kind="ExternalInput")
    w3t = nc.dram_tensor("w3t", [d, h], mybir.dt.bfloat16, kind="ExternalInput")
    w2t = nc.dram_tensor("w2t", [h, d], mybir.dt.bfloat16, kind="ExternalInput")
    outT = nc.dram_tensor("outT", [d, tpc], mybir.dt.bfloat16, kind="ExternalOutput")

    xr = xT.rearrange("(k p) t -> p k t", p=128)
    w1r = w1t.rearrange("(k p) h -> p k h", p=128)
    w3r = w3t.rearrange("(k p) h -> p k h", p=128)
    w2r = w2t.rearrange("(k p) d -> p k d", p=128)
    outr = outT.rearrange("(k p) t -> p k t", p=128)

    SILU = mybir.ActivationFunctionType.Silu
    f32 = mybir.dt.float32
    bf16 = mybir.dt.bfloat16

    with tile.TileContext(nc) as tcx:
        with (
            tcx.tile_pool(name="sx", bufs=1) as sx,
            tcx.tile_pool(name="sw", bufs=2) as sw,
            tcx.tile_pool(name="sh", bufs=kh) as sh,
            tcx.tile_pool(name="sact", bufs=3) as sact,
            tcx.tile_pool(name="sout", bufs=4) as sout,
            tcx.tile_pool(name="ps", bufs=2, space="PSUM") as ps,
        ):
            for b in range(tpc // tb):
                x_sb = sx.tile([128, kd, tb], bf16, tag="x", bufs=1, name=f"x_{b}")
                # per-ki DMAs let the first matmul chain start after 1/kd of
                # the block arrives (cuts the kernel-entry fill bubble)
                for ki in range(kd):
                    nc.sync.dma_start(x_sb[:, ki, :], xr[:, ki, b * tb : (b + 1) * tb])

                # ---- phase 1: h = silu(x@w1.T) * (x@w3.T), kept in SBUF ----
                h_tiles = []
                for hp in range(kh // 2):
                    w1_sb = sw.tile([128, kd, 256], bf16, tag="w1", bufs=2, name=f"w1_{b}_{hp}")
                    w3_sb = sw.tile([128, kd, 256], bf16, tag="w3", bufs=2, name=f"w3_{b}_{hp}")
                    nc.sync.dma_start(w1_sb[:], w1r[:, :, hp * 256 : (hp + 1) * 256])
                    nc.sync.dma_start(w3_sb[:], w3r[:, :, hp * 256 : (hp + 1) * 256])
                    for hj in range(2):
                        hi = hp * 2 + hj
                        h_sb = sh.tile([128, tb], bf16, tag="h", bufs=kh, name=f"h_{b}_{hi}")
                        for tcb in range(tb // tc):
                            ps1 = ps.tile([128, tc], f32, tag="h1", bufs=3, name=f"ps1_{b}_{hi}_{tcb}")
                            ps3 = ps.tile([128, tc], f32, tag="h3", bufs=3, name=f"ps3_{b}_{hi}_{tcb}")
                            for ki in range(kd):
                                nc.tensor.matmul(
                                    ps1[:],
                                    w1_sb[:, ki, hj * 128 : (hj + 1) * 128],
                                    x_sb[:, ki, tcb * tc : (tcb + 1) * tc],
                                    start=(ki == 0),
                                    stop=(ki == kd - 1),
                                )
                            for ki in range(kd):
                                nc.tensor.matmul(
                                    ps3[:],
                                    w3_sb[:, ki, hj * 128 : (hj + 1) * 128],
                                    x_sb[:, ki, tcb * tc : (tcb + 1) * tc],
                                    start=(ki == 0),
                                    stop=(ki == kd - 1),
                                )
                            sil = sact.tile([128, tc], f32, tag="sil", bufs=3, name=f"sil_{b}_{hi}_{tcb}")
                            nc.scalar.activation(sil[:], ps1[:], SILU)
                            nc.vector.tensor_mul(h_sb[:, tcb * tc : (tcb + 1) * tc], sil[:], ps3[:])
                        h_tiles.append(h_sb)

                # ---- phase 2: outT[dout, t] = h.T @ w2.T accumulated over h ----
                for di in range(kd):
                    w2_sb = sw.tile([128, kh, 128], bf16, tag="w2", bufs=2, name=f"w2_{b}_{di}")
                    nc.sync.dma_start(w2_sb[:], w2r[:, :, di * 128 : (di + 1) * 128])
                    for tcb in range(tb // tc):
                        pso = ps.tile([128, tc], f32, tag="o", bufs=2, name=f"pso_{b}_{di}_{tcb}")
                        for hk in range(kh):
                            nc.tensor.matmul(
                                pso[:],
                                w2_sb[:, hk, :],
                                h_tiles[hk][:, tcb * tc : (tcb + 1) * tc],
                                start=(hk == 0),
                                stop=(hk == kh - 1),
                            )
                        o_sb = sout.tile([128, tc], bf16, tag="osb", bufs=4, name=f"o_{b}_{di}_{tcb}")
                        nc.scalar.copy(o_sb[:], pso[:])
                        nc.sync.dma_start(
                            outr[:, di, b * tb + tcb * tc : b * tb + (tcb + 1) * tc],
                            o_sb[:],
                        )
    nc.compile()
    return nc


_NC = None


def _get_nc():
    global _NC
    if _NC is None:
        _NC = _build()
    return _NC


def _prep_core(args):
    """Host-side shard prep for one expert: slice+pad tokens, transpose, bf16."""
    x, w1, w3, w2, off, cnt = args
    xe = np.zeros((TPC, D), dtype=BF16)
    xe[:cnt] = x[off : off + cnt].astype(BF16)
    return {
        "xT": np.ascontiguousarray(xe.T),
        "w1t": np.ascontiguousarray(w1.T.astype(BF16)),  # [D, H]
        "w3t": np.ascontiguousarray(w3.T.astype(BF16)),  # [D, H]
        "w2t": np.ascontiguousarray(w2.T.astype(BF16)),  # [H, D]
    }


def kernel(x, w1, w2, w3, num_tokens_per_expert):
    x = np.asarray(x, dtype=np.float32)
    w1 = np.asarray(w1, dtype=np.float32)
    w2 = np.asarray(w2, dtype=np.float32)
    w3 = np.asarray(w3, dtype=np.float32)
    counts = np.asarray(num_tokens_per_expert).astype(np.int64)
    assert counts.shape == (E,) and counts.sum() == x.shape[0]
    assert counts.max() <= TPC, "per-expert shard exceeds compiled capacity"
    offs = np.concatenate([[0], np.cumsum(counts)[:-1]])

    from concurrent.futures import ThreadPoolExecutor

    with ThreadPoolExecutor(max_workers=8) as ex:
        in_maps = list(
            ex.map(
                _prep_core,
                [(x, w1[e], w3[e], w2[e], offs[e], counts[e]) for e in range(E)],
            )
        )

    nc = _get_nc()
    res = run_bass_kernel_spmd(nc, in_maps, core_ids=list(range(N_CORES)))

    out = np.empty((T, D), dtype=np.float32)

    def _post(e):
        oT = res.results[e]["outT"]  # [D, TPC] bf16
        out[offs[e] : offs[e] + counts[e]] = oT.T[: counts[e]].astype(np.float32)

    with ThreadPoolExecutor(max_workers=8) as ex:
        list(ex.map(_post, range(E)))
    return out

